# revision 1
# baseline (speedup 1.0000x reference)
"""Data-parallel Trainium kernel for nn_AttnModel3 (dense_transformer).

Strategy (per sharding hint): pure data parallel — shard sp/h1/h2 on the
batch axis across the 8 NeuronCores; all params (q/k/v kernels, norm
affines, final linear) are small and replicated. Each core runs the full
per-action attention forward for its 32-batch shard; outputs are
concatenated on the host. Compute is dispatched to the trn2 cores through
the PJRT (axon) backend with a single compiled SPMD program.
"""

import numpy as np
import jax
import jax.numpy as jnp

B, N, F = 256, 64, 64
S = 2 * N + 2  # 130
EPS = 1e-6
SCALE = float(np.sqrt(S))
NDEV = 8
BC = B // NDEV  # 32 batches per core

_COMPILED = None


def _norm(x, alpha, beta):
    m = jnp.mean(x, axis=-1, keepdims=True)
    s = jnp.std(x, axis=-1, keepdims=True)
    return alpha * (x - m) / (s + EPS) + beta


def _fwd_shard(sp, h1, h2, Wq, bq, Wk, bk, Wv, bv, a1, b1, a2, b2, Wlin, blin):
    # sp: (BC, F), h1/h2: (BC, N, F); params replicated.
    obs = jnp.concatenate((h1, h2, sp[:, None, :]), axis=1).transpose(0, 2, 1)

    def fwd_action(action):  # action: (BC, F)
        x = jnp.concatenate((obs, action[:, :, None]), axis=-1)  # (BC, F, S)
        xn = _norm(x, a1, b1)
        q = xn @ Wq + bq
        k = xn @ Wk + bk
        v = xn @ Wv + bv
        sim = jnp.einsum('bis,bjs->bij', q, k) / SCALE
        p = jax.nn.softmax(sim, axis=-1)
        ao = jnp.einsum('bij,bjs->bis', p, v)
        y = _norm(ao + ao, a2, b2)
        return y.reshape(BC, -1) @ Wlin + blin  # (BC, 1)

    qv = jax.vmap(fwd_action, in_axes=-1, out_axes=1)(h2.transpose(0, 2, 1))
    return qv.reshape(BC, N)


def _get_compiled():
    global _COMPILED
    if _COMPILED is None:
        _COMPILED = jax.pmap(
            _fwd_shard,
            axis_name='x',
            in_axes=(0, 0, 0) + (None,) * 12,
        )
    return _COMPILED


def kernel(sp, h1, h2, Wq, bq, Wk, bk, Wv, bv,
           alpha1, beta1, alpha2, beta2, Wlin, blin):
    sp_s = np.asarray(sp, np.float32).reshape(NDEV, BC, F)
    h1_s = np.asarray(h1, np.float32).reshape(NDEV, BC, N, F)
    h2_s = np.asarray(h2, np.float32).reshape(NDEV, BC, N, F)
    out = _get_compiled()(
        sp_s, h1_s, h2_s,
        Wq, bq, Wk, bk, Wv, bv,
        alpha1, beta1, alpha2, beta2, Wlin, blin,
    )
    return np.asarray(out).reshape(B, N).astype(np.float32)


if __name__ == "__main__":
    rng = np.random.default_rng(0)
    d = {
        "sp": rng.standard_normal((B, F), np.float32),
        "h1": rng.standard_normal((B, N, F), np.float32),
        "h2": rng.standard_normal((B, N, F), np.float32),
        "Wq": rng.standard_normal((S, S), np.float32) * 0.05,
        "bq": np.zeros((S,), np.float32),
        "Wk": rng.standard_normal((S, S), np.float32) * 0.05,
        "bk": np.zeros((S,), np.float32),
        "Wv": rng.standard_normal((S, S), np.float32) * 0.05,
        "bv": np.zeros((S,), np.float32),
        "alpha1": np.ones((F, S), np.float32),
        "beta1": np.zeros((F, S), np.float32),
        "alpha2": np.ones((F, S), np.float32),
        "beta2": np.zeros((F, S), np.float32),
        "Wlin": rng.standard_normal((F * S, 1), np.float32) * 0.02,
        "blin": np.zeros((1,), np.float32),
    }
    out = kernel(**d)
    print("kernel output", out.shape, out.dtype, float(np.abs(out).mean()))

